# revision 2
# baseline (speedup 1.0000x reference)
"""Trainium2 Bass kernel for nn_Covariance — fp8, 128-wide blocks, staged DMA.

PE measurements (see kernel2 probes) show the matmul moving-operand
stream is the roofline in this environment: 128-col fills run at
~92 ns regardless of LDWEIGHTS size/count, weight reuse, instruction
fusion, or PSUM ordering (~1.4 GHz effective fill clock).  260 fills
(65 blocks x 4 K-chunks) = ~24 us of PE time.  DoubleRow variants
measured worse (DR LDWEIGHTS streams at ~0.8 ns/col and cannot hide).

This kernel therefore keeps the no-perf-mode 128-wide structure and
optimizes the pipeline endpoints instead:
- input DMA staged in growing groups (8,16,16,25 blocks; swept on HW)
  so the first matmul starts after a 512 KB transfer instead of 1 MB;
- output staged in 3 pieces DMA'd on the scalar (ACT HWDGE) ring as
  soon as their copies complete, shrinking the final-DMA tail.

Per core: read 4.26 MB fp8, write 0.53 MB bf16.
"""

import numpy as np

_B, _T, _F, _M = 8, 512, 513, 8
_CH = 2 * _M            # 16 packed re/im channels per frequency
_PW = 2 * _CH           # 32 fp8 columns per frequency pair
_NP = (_F + 1) // 2     # 257 frequency pairs (pair 256 = freq 512 + zeros)
_NPP = 260              # pairs padded to a multiple of 4
_NB = _NPP // 4         # 65 blocks of 4 pairs = 128 channels
_KC = 4                 # time chunks of 128
_BW = 4 * _PW           # 128 fp8 columns per block
_NCORES = 8
_GSIZES = (8, 16, 16, 25)                # blocks per DMA group
_GPRE = np.cumsum([0] + list(_GSIZES)).tolist()
assert _GPRE[-1] == _NB
_GCOL = [_KC * n * _BW for n in _GSIZES]
_GOFF = np.cumsum([0] + _GCOL).tolist()
_TOTCOL = _GOFF[-1]     # fp8 bytes per partition row (33280)
_NQ = (_NB + 3) // 4    # 17 PSUM quads (4 blocks per PSUM bank)
_OPIECE = ((0, 8), (8, 14), (14, _NQ))   # output pieces (quad ranges)
_OCOLS = _NPP * _PW     # 8320 gram columns

_nc_cache = None


def _f8():
    import ml_dtypes

    return ml_dtypes.float8_e4m3


def _build_nc(reps=1, hw_loop=0, mode="full"):
    import contextlib

    import concourse.mybir as mybir
    from concourse import bacc, tile

    f32 = mybir.dt.float32
    bf16 = mybir.dt.bfloat16
    fp8 = mybir.dt.float8e4
    nc = bacc.Bacc(None, target_bir_lowering=False)
    hm = nc.declare_dram_parameter("hm", [128, _TOTCOL], fp8, isOutput=False)
    gram = nc.declare_dram_parameter("gram", [_PW, _OCOLS], bf16, isOutput=True)

    do_dma = mode in ("full", "nomm")
    do_mm = mode != "nomm"
    do_copies = mode != "mmnc"

    def quad_group(q):
        for g in range(len(_GSIZES)):
            if 4 * q < _GPRE[g + 1]:
                return g
        raise AssertionError

    with tile.TileContext(nc) as tc:
        with (
            tc.tile_pool(name="hm", bufs=len(_GSIZES)) as hpool,
            tc.tile_pool(name="ps", bufs=8, space="PSUM") as ppool,
            tc.tile_pool(name="out", bufs=len(_OPIECE)) as opool,
        ):
            loop_cm = (
                tc.For_i(0, hw_loop, 1,
                         hint_engines=(mybir.EngineType.PE,))
                if hw_loop else contextlib.nullcontext()
            )
            with loop_cm:
                for _rep in range(reps):
                    hts = []
                    for g, nb in enumerate(_GSIZES):
                        ht = hpool.tile([128, _KC, nb, _BW], fp8, tag="hm",
                                        name=f"ht{g}")
                        hts.append(ht)
                        if do_dma:
                            nc.sync.dma_start(
                                ht,
                                hm[:, _GOFF[g]:_GOFF[g + 1]].rearrange(
                                    "p (c j k) -> p c j k", c=_KC, j=nb
                                ),
                            )
                        else:
                            nc.sync.dma_start(
                                ht[:, 0, 0, :],
                                hm[:, _GOFF[g]:_GOFF[g] + _BW],
                            )
                    osts = [
                        opool.tile(
                            [_PW, min(_NB, q1 * 4) - q0 * 4, 4, _PW],
                            bf16, tag="o", name=f"ost{pi}")
                        for pi, (q0, q1) in enumerate(_OPIECE)
                    ]
                    for q in range(_NQ):
                        g = quad_group(q)
                        ht = hts[g]
                        ns = min(4, _NB - 4 * q)
                        pt = ppool.tile([128, 4, _BW], f32, tag="ps",
                                        name=f"pt{q}")
                        if do_mm:
                            for s in range(ns):
                                j = 4 * q + s - _GPRE[g]
                                for c in range(_KC):
                                    nc.tensor.matmul(
                                        pt[:, s, :],
                                        ht[:, c, j, :],
                                        ht[:, c, j, :],
                                        start=(c == 0),
                                        stop=(c == _KC - 1),
                                    )
                        pi = next(i for i, (q0, q1) in enumerate(_OPIECE)
                                  if q0 <= q < q1)
                        ost = osts[pi]
                        qq = q - _OPIECE[pi][0]
                        for i in range(4 if do_copies else 1):
                            rows = slice(_PW * i, _PW * (i + 1))
                            dst = ost[:, 4 * qq:4 * qq + ns, i, :]
                            if not do_copies:
                                dst = ost[0:1, 4 * qq, i, 0:1]
                                src = pt[0:1, 0, 0:1]
                            elif do_mm:
                                src = pt[rows, :ns, _PW * i:_PW * (i + 1)]
                            else:
                                j0 = 4 * q - _GPRE[g]
                                src = ht[0:_PW, i, j0:j0 + ns, 0:_PW]
                            if (q + i) % 2 == 0:
                                nc.vector.tensor_copy(dst, src)
                            else:
                                nc.scalar.copy(dst, src)
                        if q == _OPIECE[pi][1] - 1:
                            b0 = 4 * _OPIECE[pi][0]
                            b1 = min(_NB, 4 * _OPIECE[pi][1])
                            nc.scalar.dma_start(
                                gram[:, b0 * _BW:b1 * _BW].rearrange(
                                    "p (s c) -> p s c", c=_PW),
                                ost.rearrange("p a b c -> p (a b) c"),
                            )

    nc.compile()
    return nc


def _prep_hm(x2):
    """x2: [T, F*CH] fp32 -> block-packed e4m3 [128, TOTCOL]."""
    f8 = _f8()
    q = x2.astype(f8).reshape(_T, _F, _CH)
    p2 = np.zeros((_T, _NPP, _PW), dtype=f8)
    p2[:, : _F // 2, :_CH] = q[:, 0:_F - 1:2]
    p2[:, : _F // 2, _CH:] = q[:, 1:_F:2]
    p2[:, _F // 2, :_CH] = q[:, _F - 1]
    a = p2.reshape(_KC, 128, _NB, _BW)      # t = c*128 + p
    parts = []
    for g in range(len(_GSIZES)):
        blk = a[:, :, _GPRE[g]:_GPRE[g + 1], :]   # [KC, 128, nb, BW]
        parts.append(blk.transpose(1, 0, 2, 3).reshape(128, -1))
    return np.concatenate(parts, axis=1)


def _decode_gram(g):
    """g: [B, 32, OCOLS] bf16 -> C [B, F, 16, 16] (time-summed Gram)."""
    nb_ = g.shape[0]
    g = np.asarray(g, dtype=np.float32).reshape(nb_, _PW, _NPP, _PW)
    C = np.empty((nb_, _F + 1, _CH, _CH), dtype=np.float32)
    C[:, 0::2] = g[:, :_CH, : _NP, :_CH].transpose(0, 2, 1, 3)
    C[:, 1::2] = g[:, _CH:, : _NP, _CH:].transpose(0, 2, 1, 3)
    return C[:, :_F]


def kernel(Xs):
    global _nc_cache
    from concurrent.futures import ThreadPoolExecutor

    from concourse.bass_utils import run_bass_kernel_spmd

    Xs = np.asarray(Xs, dtype=np.float32)
    assert Xs.shape == (_B, _T, _F, 2, _M)
    if _nc_cache is None:
        _nc_cache = _build_nc()

    xs2 = Xs.reshape(_B, _T, _F * _CH)
    with ThreadPoolExecutor(_B) as ex:
        hms = list(ex.map(_prep_hm, [xs2[b] for b in range(_B)]))
    in_maps = [{"hm": hms[b]} for b in range(_B)]
    res = run_bass_kernel_spmd(_nc_cache, in_maps, list(range(_NCORES))).results

    C = _decode_gram(np.stack([r["gram"] for r in res]))
    iu0, iu1 = np.triu_indices(_M)
    re = C[:, :, iu0, iu1] + C[:, :, _M + iu0, _M + iu1]
    im = C[:, :, iu0, _M + iu1] - C[:, :, iu1, _M + iu0]
    mean = np.stack([re, im], axis=2) * np.float32(1.0 / _T)  # [B, F, 2, 36]
    mean = np.ascontiguousarray(mean, dtype=np.float32)
    npairs = _M * (_M + 1) // 2
    return np.broadcast_to(mean[:, None], (_B, _T, _F, 2, npairs))


# revision 3
# speedup vs baseline: 1.0221x; 1.0221x over previous
"""Trainium2 Bass kernel for nn_Covariance — fp8, 128-wide blocks, staged DMA.

PE measurements (see kernel2 probes) show the matmul moving-operand
stream is the roofline in this environment: 128-col fills run at
~92 ns regardless of LDWEIGHTS size/count, weight reuse, instruction
fusion, or PSUM ordering (~1.4 GHz effective fill clock).  260 fills
(65 blocks x 4 K-chunks) = ~24 us of PE time.  DoubleRow variants
measured worse (DR LDWEIGHTS streams at ~0.8 ns/col and cannot hide).

This kernel therefore keeps the no-perf-mode 128-wide structure and
optimizes the pipeline endpoints instead:
- input DMA staged in growing groups (8,16,16,25 blocks; swept on HW)
  so the first matmul starts after a 512 KB transfer instead of 1 MB;
- output staged in 3 pieces DMA'd on the scalar (ACT HWDGE) ring as
  soon as their copies complete, shrinking the final-DMA tail.

Per core: read 4.26 MB fp8, write 0.53 MB bf16.
"""

import numpy as np

_B, _T, _F, _M = 8, 512, 513, 8
_CH = 2 * _M            # 16 packed re/im channels per frequency
_PW = 2 * _CH           # 32 fp8 columns per frequency pair
_NP = (_F + 1) // 2     # 257 frequency pairs (pair 256 = freq 512 + zeros)
_NPP = 260              # pairs padded to a multiple of 4
_NB = _NPP // 4         # 65 blocks of 4 pairs = 128 channels
_KC = 4                 # time chunks of 128
_BW = 4 * _PW           # 128 fp8 columns per block
_NCORES = 8
_GSIZES = (8, 16, 16, 24)                # FULL blocks per DMA group
_G0SPLIT = 1                             # group-0 DMA split into chunk slices
_NBF = 64               # full 128-wide blocks; block 64 is narrow (1 pair)
_GPRE = np.cumsum([0] + list(_GSIZES)).tolist()
assert _GPRE[-1] == _NBF
_GCOL = [_KC * n * _BW for n in _GSIZES] + [_KC * _PW]
_GOFF = np.cumsum([0] + _GCOL).tolist()
_TOTCOL = _GOFF[-1]     # fp8 bytes per partition row (32896)
_NQ = (_NB + 3) // 4    # 17 PSUM quads (4 blocks per PSUM bank)
_OPIECE = ((0, 8), (8, 14), (14, _NQ))   # output pieces (quad ranges)
_OCOLS = _NPP * _PW     # 8320 gram columns

_nc_cache = None


def _f8():
    import ml_dtypes

    return ml_dtypes.float8_e4m3


def _build_nc(reps=1, hw_loop=0, mode="full"):
    import contextlib

    import concourse.mybir as mybir
    from concourse import bacc, tile

    f32 = mybir.dt.float32
    bf16 = mybir.dt.bfloat16
    fp8 = mybir.dt.float8e4
    nc = bacc.Bacc(None, target_bir_lowering=False)
    hm = nc.declare_dram_parameter("hm", [128, _TOTCOL], fp8, isOutput=False)
    gram = nc.declare_dram_parameter("gram", [_PW, _OCOLS], bf16, isOutput=True)

    do_dma = mode in ("full", "nomm")
    do_mm = mode != "nomm"
    do_copies = mode not in ("mmnc", "mmgf")

    def quad_group(q):
        for g in range(len(_GSIZES)):
            if 4 * q < _GPRE[g + 1]:
                return g
        raise AssertionError

    with tile.TileContext(nc) as tc:
        with (
            tc.tile_pool(name="hm", bufs=len(_GSIZES) + 1) as hpool,
            tc.tile_pool(name="ps", bufs=8, space="PSUM") as ppool,
            tc.tile_pool(name="out", bufs=len(_OPIECE)) as opool,
        ):
            loop_cm = (
                tc.For_i(0, hw_loop, 1,
                         hint_engines=(mybir.EngineType.PE,))
                if hw_loop else contextlib.nullcontext()
            )
            with loop_cm:
                for _rep in range(reps):
                    if mode == "mmgf":
                        # gap-free PE probe: every MM hits one PSUM tile,
                        # no copies, sliver DMAs
                        ht = hpool.tile([128, _KC, _NB, _BW], fp8, tag="hm",
                                        name="htg")
                        nc.sync.dma_start(ht[:, 0, 0, :], hm[:, 0:_BW])
                        pt = ppool.tile([128, 4, _BW], f32, tag="ps",
                                        name="ptg")
                        for b in range(_NB):
                            for c in range(_KC):
                                nc.tensor.matmul(
                                    pt[:, 0, :], ht[:, c, b, :],
                                    ht[:, c, b, :], start=True, stop=True,
                                )
                        ost = opool.tile([_PW, 4, _PW], bf16, tag="o",
                                         name="ostg")
                        nc.vector.tensor_copy(ost[0:1, 0, 0:1], pt[0:1, 0, 0:1])
                        nc.scalar.dma_start(gram[0:1, 0:_PW],
                                            ost[0:1, 0, :])
                        continue
                    hts = []
                    for g, nb in enumerate(_GSIZES):
                        ht = hpool.tile([128, _KC, nb, _BW], fp8, tag="hm",
                                        name=f"ht{g}")
                        hts.append(ht)
                        if do_dma:
                            nc.sync.dma_start(
                                ht,
                                hm[:, _GOFF[g]:_GOFF[g + 1]].rearrange(
                                    "p (c j k) -> p c j k", c=_KC, j=nb
                                ),
                            )
                        else:
                            nc.sync.dma_start(
                                ht[:, 0, 0, :],
                                hm[:, _GOFF[g]:_GOFF[g] + _BW],
                            )
                    htn = hpool.tile([128, _KC, _PW], fp8, tag="hm",
                                     name="htn")
                    nc.sync.dma_start(
                        htn,
                        hm[:, _GOFF[-2]:_GOFF[-1]].rearrange(
                            "p (c k) -> p c k", c=_KC),
                    )
                    osts = [
                        opool.tile(
                            [_PW, min(_NB, q1 * 4) - q0 * 4, 4, _PW],
                            bf16, tag="o", name=f"ost{pi}")
                        for pi, (q0, q1) in enumerate(_OPIECE)
                    ]
                    for q in range(_NQ):
                        pt = ppool.tile([128, 4, _BW], f32, tag="ps",
                                        name=f"pt{q}")
                        if q == _NQ - 1:
                            # narrow final block: 1 real pair, 32 cols
                            pi = len(_OPIECE) - 1
                            ost = osts[pi]
                            qq = q - _OPIECE[pi][0]
                            if do_mm:
                                for c in range(_KC):
                                    nc.tensor.matmul(
                                        pt[:_PW, 0, :_PW],
                                        htn[:, c, :],
                                        htn[:, c, :],
                                        start=(c == 0),
                                        stop=(c == _KC - 1),
                                    )
                                src = pt[:_PW, 0, :_PW]
                            else:
                                src = htn[0:_PW, 0, :]
                            if not do_copies:
                                nc.vector.tensor_copy(
                                    ost[0:1, 4 * qq, 0, 0:1], pt[0:1, 0, 0:1])
                            else:
                                nc.vector.tensor_copy(
                                    ost[:, 4 * qq, 0, :], src)
                            b0 = 4 * _OPIECE[pi][0]
                            nc.scalar.dma_start(
                                gram[:, b0 * _BW:_OCOLS].rearrange(
                                    "p (s c) -> p s c", c=_PW),
                                ost.rearrange("p a b c -> p (a b) c"),
                            )
                            continue
                        g = quad_group(q)
                        ht = hts[g]
                        ns = min(4, _NBF - 4 * q)
                        if do_mm:
                            # chunk-outer in (split) group 0 so the first
                            # matmuls only need the first chunk slice
                            order = (
                                [(s, c) for c in range(_KC)
                                 for s in range(ns)]
                                if g == 0 and _G0SPLIT > 1 else
                                [(s, c) for s in range(ns)
                                 for c in range(_KC)]
                            )
                            for s, c in order:
                                j = 4 * q + s - _GPRE[g]
                                nc.tensor.matmul(
                                    pt[:, s, :],
                                    ht[:, c, j, :],
                                    ht[:, c, j, :],
                                    start=(c == 0),
                                    stop=(c == _KC - 1),
                                )
                        pi = next(i for i, (q0, q1) in enumerate(_OPIECE)
                                  if q0 <= q < q1)
                        ost = osts[pi]
                        qq = q - _OPIECE[pi][0]
                        for i in range(4 if do_copies else 1):
                            rows = slice(_PW * i, _PW * (i + 1))
                            dst = ost[:, 4 * qq:4 * qq + ns, i, :]
                            if not do_copies:
                                dst = ost[0:1, 4 * qq, i, 0:1]
                                src = pt[0:1, 0, 0:1]
                            elif do_mm:
                                src = pt[rows, :ns, _PW * i:_PW * (i + 1)]
                            else:
                                j0 = 4 * q - _GPRE[g]
                                src = ht[0:_PW, i, j0:j0 + ns, 0:_PW]
                            if (q + i) % 2 == 0:
                                nc.vector.tensor_copy(dst, src)
                            else:
                                nc.scalar.copy(dst, src)
                        if q == _OPIECE[pi][1] - 1:
                            b0 = 4 * _OPIECE[pi][0]
                            b1 = min(_NB, 4 * _OPIECE[pi][1])
                            nc.scalar.dma_start(
                                gram[:, b0 * _BW:b1 * _BW].rearrange(
                                    "p (s c) -> p s c", c=_PW),
                                ost.rearrange("p a b c -> p (a b) c"),
                            )

    nc.compile()
    return nc


def _prep_hm(x2):
    """x2: [T, F*CH] fp32 -> block-packed e4m3 [128, TOTCOL]."""
    f8 = _f8()
    q = x2.astype(f8).reshape(_T, _F, _CH)
    p2 = np.zeros((_T, _NPP, _PW), dtype=f8)
    p2[:, : _F // 2, :_CH] = q[:, 0:_F - 1:2]
    p2[:, : _F // 2, _CH:] = q[:, 1:_F:2]
    p2[:, _F // 2, :_CH] = q[:, _F - 1]
    a = p2[:, :4 * _NBF].reshape(_KC, 128, _NBF, _BW)   # t = c*128 + p
    parts = []
    for g in range(len(_GSIZES)):
        blk = a[:, :, _GPRE[g]:_GPRE[g + 1], :]   # [KC, 128, nb, BW]
        parts.append(blk.transpose(1, 0, 2, 3).reshape(128, -1))
    an = p2[:, 4 * _NBF].reshape(_KC, 128, _PW)   # narrow last pair
    parts.append(an.transpose(1, 0, 2).reshape(128, -1))
    return np.concatenate(parts, axis=1)


def _decode_gram(g):
    """g: [B, 32, OCOLS] bf16 -> C [B, F, 16, 16] (time-summed Gram)."""
    nb_ = g.shape[0]
    g = np.asarray(g, dtype=np.float32).reshape(nb_, _PW, _NPP, _PW)
    C = np.empty((nb_, _F + 1, _CH, _CH), dtype=np.float32)
    C[:, 0::2] = g[:, :_CH, : _NP, :_CH].transpose(0, 2, 1, 3)
    C[:, 1::2] = g[:, _CH:, : _NP, _CH:].transpose(0, 2, 1, 3)
    return C[:, :_F]


def kernel(Xs):
    global _nc_cache
    from concurrent.futures import ThreadPoolExecutor

    from concourse.bass_utils import run_bass_kernel_spmd

    Xs = np.asarray(Xs, dtype=np.float32)
    assert Xs.shape == (_B, _T, _F, 2, _M)
    if _nc_cache is None:
        _nc_cache = _build_nc()

    xs2 = Xs.reshape(_B, _T, _F * _CH)
    with ThreadPoolExecutor(_B) as ex:
        hms = list(ex.map(_prep_hm, [xs2[b] for b in range(_B)]))
    in_maps = [{"hm": hms[b]} for b in range(_B)]
    res = run_bass_kernel_spmd(_nc_cache, in_maps, list(range(_NCORES))).results

    C = _decode_gram(np.stack([r["gram"] for r in res]))
    iu0, iu1 = np.triu_indices(_M)
    re = C[:, :, iu0, iu1] + C[:, :, _M + iu0, _M + iu1]
    im = C[:, :, iu0, _M + iu1] - C[:, :, iu1, _M + iu0]
    mean = np.stack([re, im], axis=2) * np.float32(1.0 / _T)  # [B, F, 2, 36]
    mean = np.ascontiguousarray(mean, dtype=np.float32)
    npairs = _M * (_M + 1) // 2
    return np.broadcast_to(mean[:, None], (_B, _T, _F, 2, npairs))


# revision 4
# speedup vs baseline: 1.0343x; 1.0120x over previous
"""Trainium2 Bass kernel for nn_Covariance — fp8, 128-wide blocks, staged DMA.

PE measurements (see kernel2 probes) show the matmul moving-operand
stream is the roofline in this environment: 128-col fills run at
~92 ns regardless of LDWEIGHTS size/count, weight reuse, instruction
fusion, or PSUM ordering (~1.4 GHz effective fill clock).  260 fills
(65 blocks x 4 K-chunks) = ~24 us of PE time.  DoubleRow variants
measured worse (DR LDWEIGHTS streams at ~0.8 ns/col and cannot hide).

This kernel therefore keeps the no-perf-mode 128-wide structure and
optimizes the pipeline endpoints instead:
- input DMA staged in growing groups (8,16,16,24 full blocks + one
  narrow block; swept on HW) so the first matmul starts after a 512 KB
  transfer instead of 1 MB;
- output staged in 3 pieces DMA'd on the scalar (ACT HWDGE) ring as
  soon as their copies complete, shrinking the final-DMA tail;
- the 65th block (1 real pair + 3 zero pads) runs as a narrow 32-col
  matmul, trimming padded fills and the tail-quad copies.

Per core: read 4.21 MB fp8, write 0.53 MB bf16.
"""

import numpy as np

_B, _T, _F, _M = 8, 512, 513, 8
_CH = 2 * _M            # 16 packed re/im channels per frequency
_PW = 2 * _CH           # 32 fp8 columns per frequency pair
_NP = (_F + 1) // 2     # 257 frequency pairs (pair 256 = freq 512 + zeros)
_NPP = 260              # pairs padded to a multiple of 4
_NB = _NPP // 4         # 65 blocks of 4 pairs = 128 channels
_KC = 4                 # time chunks of 128
_BW = 4 * _PW           # 128 fp8 columns per block
_NCORES = 8
_GSIZES = (8, 16, 16, 24)                # FULL blocks per DMA group
_G0SPLIT = 1                             # group-0 DMA split into chunk slices
_NBF = 64               # full 128-wide blocks; block 64 is narrow (1 pair)
_GPRE = np.cumsum([0] + list(_GSIZES)).tolist()
assert _GPRE[-1] == _NBF
_GCOL = [_KC * n * _BW for n in _GSIZES] + [_KC * _PW]
_GOFF = np.cumsum([0] + _GCOL).tolist()
_TOTCOL = _GOFF[-1]     # fp8 bytes per partition row (32896)
_NQ = (_NB + 3) // 4    # 17 PSUM quads (4 blocks per PSUM bank)
_OPIECE = ((0, 8), (8, 14), (14, _NQ))   # output pieces (quad ranges)
_OCOLS = _NPP * _PW     # 8320 gram columns

_nc_cache = None


def _f8():
    import ml_dtypes

    return ml_dtypes.float8_e4m3


def _build_nc(reps=1, hw_loop=0, mode="full"):
    import contextlib

    import concourse.mybir as mybir
    from concourse import bacc, tile

    f32 = mybir.dt.float32
    bf16 = mybir.dt.bfloat16
    fp8 = mybir.dt.float8e4
    nc = bacc.Bacc(None, target_bir_lowering=False)
    hm = nc.declare_dram_parameter("hm", [128, _TOTCOL], fp8, isOutput=False)
    gram = nc.declare_dram_parameter("gram", [_PW, _OCOLS], bf16, isOutput=True)

    do_dma = mode in ("full", "nomm")
    do_mm = mode != "nomm"
    do_copies = mode not in ("mmnc", "mmgf")

    def quad_group(q):
        for g in range(len(_GSIZES)):
            if 4 * q < _GPRE[g + 1]:
                return g
        raise AssertionError

    with tile.TileContext(nc) as tc:
        with (
            tc.tile_pool(name="hm", bufs=len(_GSIZES) + 1) as hpool,
            tc.tile_pool(name="ps", bufs=8, space="PSUM") as ppool,
            tc.tile_pool(name="out", bufs=len(_OPIECE)) as opool,
        ):
            loop_cm = (
                tc.For_i(0, hw_loop, 1,
                         hint_engines=(mybir.EngineType.PE,))
                if hw_loop else contextlib.nullcontext()
            )
            with loop_cm:
                for _rep in range(reps):
                    if mode == "mmgf":
                        # gap-free PE probe: every MM hits one PSUM tile,
                        # no copies, sliver DMAs
                        ht = hpool.tile([128, _KC, _NB, _BW], fp8, tag="hm",
                                        name="htg")
                        nc.sync.dma_start(ht[:, 0, 0, :], hm[:, 0:_BW])
                        pt = ppool.tile([128, 4, _BW], f32, tag="ps",
                                        name="ptg")
                        for b in range(_NB):
                            for c in range(_KC):
                                nc.tensor.matmul(
                                    pt[:, 0, :], ht[:, c, b, :],
                                    ht[:, c, b, :], start=True, stop=True,
                                )
                        ost = opool.tile([_PW, 4, _PW], bf16, tag="o",
                                         name="ostg")
                        nc.vector.tensor_copy(ost[0:1, 0, 0:1], pt[0:1, 0, 0:1])
                        nc.scalar.dma_start(gram[0:1, 0:_PW],
                                            ost[0:1, 0, :])
                        continue
                    hts = []
                    for g, nb in enumerate(_GSIZES):
                        ht = hpool.tile([128, _KC, nb, _BW], fp8, tag="hm",
                                        name=f"ht{g}")
                        hts.append(ht)
                        if do_dma:
                            nc.sync.dma_start(
                                ht,
                                hm[:, _GOFF[g]:_GOFF[g + 1]].rearrange(
                                    "p (c j k) -> p c j k", c=_KC, j=nb
                                ),
                            )
                        else:
                            nc.sync.dma_start(
                                ht[:, 0, 0, :],
                                hm[:, _GOFF[g]:_GOFF[g] + _BW],
                            )
                    htn = hpool.tile([128, _KC, _PW], fp8, tag="hm",
                                     name="htn")
                    nc.sync.dma_start(
                        htn,
                        hm[:, _GOFF[-2]:_GOFF[-1]].rearrange(
                            "p (c k) -> p c k", c=_KC),
                    )
                    osts = [
                        opool.tile(
                            [_PW, min(_NB, q1 * 4) - q0 * 4, 4, _PW],
                            bf16, tag="o", name=f"ost{pi}")
                        for pi, (q0, q1) in enumerate(_OPIECE)
                    ]
                    for q in range(_NQ):
                        pt = ppool.tile([128, 4, _BW], f32, tag="ps",
                                        name=f"pt{q}")
                        if q == _NQ - 1:
                            # narrow final block: 1 real pair, 32 cols
                            pi = len(_OPIECE) - 1
                            ost = osts[pi]
                            qq = q - _OPIECE[pi][0]
                            if do_mm:
                                for c in range(_KC):
                                    nc.tensor.matmul(
                                        pt[:_PW, 0, :_PW],
                                        htn[:, c, :],
                                        htn[:, c, :],
                                        start=(c == 0),
                                        stop=(c == _KC - 1),
                                    )
                                src = pt[:_PW, 0, :_PW]
                            else:
                                src = htn[0:_PW, 0, :]
                            if not do_copies:
                                nc.vector.tensor_copy(
                                    ost[0:1, 4 * qq, 0, 0:1], pt[0:1, 0, 0:1])
                            else:
                                nc.vector.tensor_copy(
                                    ost[:, 4 * qq, 0, :], src)
                            b0 = 4 * _OPIECE[pi][0]
                            nc.scalar.dma_start(
                                gram[:, b0 * _BW:_OCOLS].rearrange(
                                    "p (s c) -> p s c", c=_PW),
                                ost.rearrange("p a b c -> p (a b) c"),
                            )
                            continue
                        g = quad_group(q)
                        ht = hts[g]
                        ns = min(4, _NBF - 4 * q)
                        if do_mm:
                            # chunk-outer in (split) group 0 so the first
                            # matmuls only need the first chunk slice
                            order = (
                                [(s, c) for c in range(_KC)
                                 for s in range(ns)]
                                if g == 0 and _G0SPLIT > 1 else
                                [(s, c) for s in range(ns)
                                 for c in range(_KC)]
                            )
                            for s, c in order:
                                j = 4 * q + s - _GPRE[g]
                                nc.tensor.matmul(
                                    pt[:, s, :],
                                    ht[:, c, j, :],
                                    ht[:, c, j, :],
                                    start=(c == 0),
                                    stop=(c == _KC - 1),
                                )
                        pi = next(i for i, (q0, q1) in enumerate(_OPIECE)
                                  if q0 <= q < q1)
                        ost = osts[pi]
                        qq = q - _OPIECE[pi][0]
                        for i in range(4 if do_copies else 1):
                            rows = slice(_PW * i, _PW * (i + 1))
                            dst = ost[:, 4 * qq:4 * qq + ns, i, :]
                            if not do_copies:
                                dst = ost[0:1, 4 * qq, i, 0:1]
                                src = pt[0:1, 0, 0:1]
                            elif do_mm:
                                src = pt[rows, :ns, _PW * i:_PW * (i + 1)]
                            else:
                                j0 = 4 * q - _GPRE[g]
                                src = ht[0:_PW, i, j0:j0 + ns, 0:_PW]
                            if (q + i) % 2 == 0:
                                nc.vector.tensor_copy(dst, src)
                            else:
                                nc.scalar.copy(dst, src)
                        if q == _OPIECE[pi][1] - 1:
                            b0 = 4 * _OPIECE[pi][0]
                            b1 = min(_NB, 4 * _OPIECE[pi][1])
                            nc.scalar.dma_start(
                                gram[:, b0 * _BW:b1 * _BW].rearrange(
                                    "p (s c) -> p s c", c=_PW),
                                ost.rearrange("p a b c -> p (a b) c"),
                            )

    nc.compile()
    return nc


def _prep_hm(x2):
    """x2: [T, F*CH] fp32 -> block-packed e4m3 [128, TOTCOL]."""
    f8 = _f8()
    q = x2.astype(f8).reshape(_T, _F, _CH)
    p2 = np.zeros((_T, _NPP, _PW), dtype=f8)
    p2[:, : _F // 2, :_CH] = q[:, 0:_F - 1:2]
    p2[:, : _F // 2, _CH:] = q[:, 1:_F:2]
    p2[:, _F // 2, :_CH] = q[:, _F - 1]
    a = p2[:, :4 * _NBF].reshape(_KC, 128, _NBF, _BW)   # t = c*128 + p
    parts = []
    for g in range(len(_GSIZES)):
        blk = a[:, :, _GPRE[g]:_GPRE[g + 1], :]   # [KC, 128, nb, BW]
        parts.append(blk.transpose(1, 0, 2, 3).reshape(128, -1))
    an = p2[:, 4 * _NBF].reshape(_KC, 128, _PW)   # narrow last pair
    parts.append(an.transpose(1, 0, 2).reshape(128, -1))
    return np.concatenate(parts, axis=1)


def _decode_gram(g):
    """g: [B, 32, OCOLS] bf16 -> C [B, F, 16, 16] (time-summed Gram)."""
    nb_ = g.shape[0]
    g = np.asarray(g, dtype=np.float32).reshape(nb_, _PW, _NPP, _PW)
    C = np.empty((nb_, _F + 1, _CH, _CH), dtype=np.float32)
    C[:, 0::2] = g[:, :_CH, : _NP, :_CH].transpose(0, 2, 1, 3)
    C[:, 1::2] = g[:, _CH:, : _NP, _CH:].transpose(0, 2, 1, 3)
    return C[:, :_F]


def kernel(Xs):
    global _nc_cache
    from concurrent.futures import ThreadPoolExecutor

    from concourse.bass_utils import run_bass_kernel_spmd

    Xs = np.asarray(Xs, dtype=np.float32)
    assert Xs.shape == (_B, _T, _F, 2, _M)
    if _nc_cache is None:
        _nc_cache = _build_nc()

    xs2 = Xs.reshape(_B, _T, _F * _CH)
    with ThreadPoolExecutor(_B) as ex:
        hms = list(ex.map(_prep_hm, [xs2[b] for b in range(_B)]))
    in_maps = [{"hm": hms[b]} for b in range(_B)]
    res = run_bass_kernel_spmd(_nc_cache, in_maps, list(range(_NCORES))).results

    C = _decode_gram(np.stack([r["gram"] for r in res]))
    iu0, iu1 = np.triu_indices(_M)
    re = C[:, :, iu0, iu1] + C[:, :, _M + iu0, _M + iu1]
    im = C[:, :, iu0, _M + iu1] - C[:, :, iu1, _M + iu0]
    mean = np.stack([re, im], axis=2) * np.float32(1.0 / _T)  # [B, F, 2, 36]
    mean = np.ascontiguousarray(mean, dtype=np.float32)
    npairs = _M * (_M + 1) // 2
    return np.broadcast_to(mean[:, None], (_B, _T, _F, 2, npairs))
